# revision 5
# baseline (speedup 1.0000x reference)
"""MeanAggregator (GNN message passing) Trainium2 Bass kernel.

Reference computation:
    neigh_idx = concat([neighbours, nodes[:, None]], axis=1)   # [B, K+1]
    out = features[neigh_idx].mean(axis=1)                     # [B, D]

Strategy: data-parallel over the 8 NeuronCores. Each core handles B/8 nodes;
the feature table is replicated. On-device, blocks of 128 nodes are processed
with one indirect (gather) DMA per block — 128x11 = 1408 rows of 512 B each —
followed by a 5-op in-place tree reduction on the vector engine and a
scale-by-1/11 copy on the scalar engine, then a store.
"""

import numpy as np

B = 100000
K = 10
KP1 = K + 1
N = 1000000
D = 128
NCORES = 8
BPC = B // NCORES          # 12500 nodes per core
P = 128                    # partitions / nodes per block
NBLK = (BPC + P - 1) // P  # 98 blocks
PAD = NBLK * P             # 12544 padded nodes per core

_CACHE = {}


def build_nc(nblk=NBLK, n_rows=N, gather_bufs=8, num_swdge_queues=1):
    """Build + compile the per-core Bass program (SPMD: same NEFF on all cores)."""
    import concourse.bacc as bacc
    import concourse.bass as bass
    import concourse.mybir as mybir
    import concourse.tile as tile

    nc = bacc.Bacc(
        "TRN2",
        target_bir_lowering=False,
        debug=False,
        num_devices=NCORES,
        num_swdge_queues=num_swdge_queues,
    )
    feat = nc.dram_tensor("features", [n_rows, D], mybir.dt.float32, kind="ExternalInput")
    idx = nc.dram_tensor("idx", [P, nblk * KP1], mybir.dt.int32, kind="ExternalInput")
    out = nc.dram_tensor("out", [nblk * P, D], mybir.dt.float32, kind="ExternalOutput")

    with tile.TileContext(nc) as tc:
        with (
            tc.tile_pool(name="idxp", bufs=1) as idxp,
            tc.tile_pool(name="gp", bufs=gather_bufs) as gp,
            tc.tile_pool(name="op", bufs=gather_bufs) as op_,
        ):
            idx_sb = idxp.tile([P, nblk * KP1], mybir.dt.int32)
            nc.sync.dma_start(out=idx_sb[:], in_=idx.ap())
            qi = 0
            for b in range(nblk):
                g = gp.tile([P, KP1 * D], mybir.dt.float32, name=f"g{b}", tag="g")
                # HW indirect DMA honors exactly one index per partition, so
                # gather the 11 neighbour rows with 11 narrow gathers.
                for j in range(KP1):
                    inst = nc.gpsimd.indirect_dma_start(
                        out=g[:, j * D:(j + 1) * D],
                        out_offset=None,
                        in_=feat.ap(),
                        in_offset=bass.IndirectOffsetOnAxis(
                            ap=idx_sb[:, b * KP1 + j:b * KP1 + j + 1], axis=0
                        ),
                    )
                    if num_swdge_queues > 1:
                        q = qi % num_swdge_queues
                        qi += 1
                        if q:
                            inst.ins.queue = f"qPoolDynamic{q}"
                gf = g[:]
                # Tree-reduce the 11 chunks of 128 floats into chunk 0.
                nc.vector.tensor_add(out=gf[:, 0:640], in0=gf[:, 0:640], in1=gf[:, 640:1280])
                nc.vector.tensor_add(out=gf[:, 0:256], in0=gf[:, 0:256], in1=gf[:, 256:512])
                nc.vector.tensor_add(out=gf[:, 0:128], in0=gf[:, 0:128], in1=gf[:, 128:256])
                nc.vector.tensor_add(out=gf[:, 0:128], in0=gf[:, 0:128], in1=gf[:, 512:640])
                nc.vector.tensor_add(out=gf[:, 0:128], in0=gf[:, 0:128], in1=gf[:, 1280:1408])
                o = op_.tile([P, D], mybir.dt.float32, name=f"o{b}", tag="o")
                nc.scalar.activation(
                    out=o[:],
                    in_=gf[:, 0:128],
                    func=mybir.ActivationFunctionType.Copy,
                    scale=1.0 / KP1,
                )
                nc.sync.dma_start(out=out.ap()[b * P:(b + 1) * P, :], in_=o[:])
    nc.compile()
    return nc


def _prep_idx(idx_rows, nblk=NBLK):
    """[rows, 11] int32 -> SBUF layout [128, nblk*11] (partition-major blocks)."""
    pad = nblk * P
    padded = np.zeros((pad, KP1), np.int32)
    padded[: idx_rows.shape[0]] = idx_rows
    return np.ascontiguousarray(
        padded.reshape(nblk, P, KP1).transpose(1, 0, 2)
    ).reshape(P, nblk * KP1)


def prep_inputs(idx_rows, nblk=NBLK, n_rows=N):
    """Per-core input map (minus features) for a core handling idx_rows."""
    return {"idx": _prep_idx(idx_rows, nblk=nblk)}


def extract_out(out_arr, nblk=NBLK):
    return out_arr


def build_in_maps(inputs):
    nodes = np.asarray(inputs["nodes"])
    neighbours = np.asarray(inputs["neighbours"])
    features = np.ascontiguousarray(np.asarray(inputs["features"], dtype=np.float32))
    idx_all = np.empty((B, KP1), np.int32)
    idx_all[:, :K] = neighbours
    idx_all[:, K] = nodes
    return [
        {"features": features, "idx": _prep_idx(idx_all[c * BPC:(c + 1) * BPC])}
        for c in range(NCORES)
    ]


def kernel(nodes, neighbours, features):
    from concourse.bass_utils import run_bass_kernel_spmd

    nodes = np.asarray(nodes)
    neighbours = np.asarray(neighbours)
    features = np.ascontiguousarray(np.asarray(features, dtype=np.float32))

    idx_all = np.empty((B, KP1), np.int32)
    idx_all[:, :K] = neighbours
    idx_all[:, K] = nodes

    if "nc" not in _CACHE:
        _CACHE["nc"] = build_nc()
    nc = _CACHE["nc"]

    in_maps = build_in_maps(
        {"nodes": nodes, "neighbours": neighbours, "features": features}
    )
    res = run_bass_kernel_spmd(nc, in_maps, core_ids=list(range(NCORES)))
    return np.concatenate([res.results[c]["out"][:BPC] for c in range(NCORES)], axis=0)
